# revision 1
# baseline (speedup 1.0000x reference)
"""Trainium2 Bass kernel for nn_Baseline_SelfGCN (gnn_message_passing).

Data-parallel over batch: 8 NeuronCores x 8 images each. Each core:
  - GAP of x_global + BN(gb)                          -> out[:, 0:2048]
  - mask downsample -> onehot -> counts/drop logic
  - segment-sum pooling of x_gcn (PE transpose + onehot matmul); the
    1/count scaling is applied after the (linear) layer-1 matmul
  - 2-layer GCN (x@W -> blockdiag(adjT)@s + bias -> BN -> relu); the self
    branch shares layer-1 x@W1 (self_feat = mask_feat with the dropped
    part's row zeroed, so s_self = rowmask * s)
  - mean over parts + BN(gn)                          -> out[:, 2048:6144]
  - concat features                                   -> out[:, 6144:43008]

Large matmuls run in float32r (full-rate PE; inputs rounded at producing
ops / cast-DMA). Mean/counts/drop-logic matmuls and transposes stay fp32.

Self-contained: hardcodes shapes; host side only shards/gathers.
"""

import numpy as np

import concourse.bass as bass
import concourse.mybir as mybir
import concourse.tile as tile
from concourse.masks import make_identity

F32 = mybir.dt.float32
F32R = mybir.dt.float32r
I32 = mybir.dt.int32
AF = mybir.ActivationFunctionType
OP = mybir.AluOpType

BL = 8          # images per core
C = 2048
HW = 256        # Hf*Wf
NPARTS = 9      # graph nodes (parts 1..9)
R = BL * NPARTS  # 72 rows = (image, part)
EPS = 1e-5
NCH = 4         # 2048 / 512 N-chunks
KT = 16         # 2048 / 128 K-tiles
OUTW = 3 * C + 2 * NPARTS * C  # 43008

MM_FAST = True  # float32r for the big matmuls
DTM = F32R if MM_FAST else F32


def legalize_waits(nc, max_waits=1):
    """Split multi-wait instructions: this walrus build allows only one
    embedded sync-wait per instruction; hoist extras onto standalone
    InstEventSemaphore waits on the same engine."""
    cnt = 0
    for fn in nc.m.functions:
        for blk in fn.blocks:
            out = []
            for inst in blk.instructions:
                si = inst.sync_info
                if si is not None and si.on_wait and len(si.on_wait) > max_waits:
                    waits = list(si.on_wait)
                    for w in waits[:-max_waits]:
                        cnt += 1
                        wi = mybir.InstEventSemaphore(
                            name=f"wsplit{cnt}_{inst.name}", ins=[], outs=[],
                            sync_info=mybir.SyncInfo(on_wait=[w], on_update=[]))
                        wi.engine = inst.engine
                        nc.register_instruction(wi)
                        out.append(wi)
                    si.on_wait = waits[-max_waits:]
                    inst.sync_info = si
                out.append(inst)
            blk.instructions = out
    return cnt


def _bcast_part(ap, n):
    """Broadcast a DRAM AP across n leading partitions (step-0 dim)."""
    return bass.AP(tensor=ap.tensor, offset=ap.offset, ap=[[0, n]] + list(ap.ap))


def build_bass():
    nc = bass.Bass()

    xg_p = nc.declare_dram_parameter("x_global", [BL, C, HW], F32, isOutput=False)
    xc_p = nc.declare_dram_parameter("x_gcn", [BL, C, HW], F32, isOutput=False)
    mk_p = nc.declare_dram_parameter("mask", [BL, 256, 256], I32, isOutput=False)
    adj_p = nc.declare_dram_parameter("adj", [BL, NPARTS, NPARTS], F32, isOutput=False)
    w1_p = nc.declare_dram_parameter("W1", [C, C], F32, isOutput=False)
    w2_p = nc.declare_dram_parameter("W2", [C, C], F32, isOutput=False)
    b1_p = nc.declare_dram_parameter("b1", [C], F32, isOutput=False)
    b2_p = nc.declare_dram_parameter("b2", [C], F32, isOutput=False)
    bn1 = {k: nc.declare_dram_parameter(k + "1", [NPARTS * C], F32, isOutput=False)
           for k in ("g", "be", "rm", "rv")}
    bn2 = {k: nc.declare_dram_parameter(k + "2", [NPARTS * C], F32, isOutput=False)
           for k in ("g", "be", "rm", "rv")}
    gb = {k: nc.declare_dram_parameter("gb_" + k, [C], F32, isOutput=False)
          for k in ("g", "b", "rm", "rv")}
    gn = {k: nc.declare_dram_parameter("gn_" + k, [C], F32, isOutput=False)
          for k in ("g", "b", "rm", "rv")}
    out_p = nc.declare_dram_parameter("out", [BL, OUTW], F32, isOutput=True)

    with tile.TileContext(nc) as tc:
        with (
            tc.tile_pool(name="consts", bufs=1) as cs,
            tc.tile_pool(name="ps", bufs=8, space="PSUM") as ps,
            tc.tile_pool(name="dram", bufs=1, space="DRAM") as dp,
        ):
            # ---------------- constants ----------------
            ident = cs.tile([128, 128], F32)
            make_identity(nc, ident[:])

            iota_i = cs.tile([128, NPARTS], I32)
            nc.gpsimd.iota(iota_i[:], pattern=[[1, NPARTS]], base=1, channel_multiplier=0)
            iota_f = cs.tile([128, NPARTS], F32)
            nc.vector.tensor_copy(out=iota_f[:], in_=iota_i[:])

            ones_col = cs.tile([128, 1], F32)
            nc.vector.memset(ones_col[:], 1.0)

            # strictly-lower-triangular L9: L[q,p] = 1 if q < p
            L9 = cs.tile([NPARTS, NPARTS], F32)
            nc.gpsimd.memset(L9[:], 0.0)
            nc.gpsimd.affine_select(
                out=L9[:], in_=L9[:], compare_op=OP.is_ge, fill=1.0,
                base=0, pattern=[[-1, NPARTS]], channel_multiplier=1,
            )

            # block "mean over parts" matrix (72, 8): 1/9 on image blocks
            # band-select: onesblk[r, b] = 1/9 iff 0 <= r - 9b <= 8
            onesblk = cs.tile([R, BL], F32)
            nc.gpsimd.memset(onesblk[:], 1.0 / NPARTS)
            nc.gpsimd.affine_select(
                out=onesblk[:], in_=onesblk[:], compare_op=OP.is_ge, fill=0.0,
                base=0, pattern=[[-NPARTS, BL]], channel_multiplier=1)
            nc.gpsimd.affine_select(
                out=onesblk[:], in_=onesblk[:], compare_op=OP.is_ge, fill=0.0,
                base=NPARTS - 1, pattern=[[NPARTS, BL]], channel_multiplier=-1)

            sgn = cs.tile([BL, C], F32)
            tgn = cs.tile([BL, C], F32)
            sgb = cs.tile([128, KT], F32)
            tgb = cs.tile([128, KT], F32)

            # DRAM scratch
            scr = {li: (dp.tile([NPARTS, C], F32, name=f"scr_s{li}"),
                        dp.tile([NPARTS, C], F32, name=f"scr_t{li}"))
                   for li in (0, 1)}
            scr_sm = dp.tile([NPARTS, BL], F32, name="scr_sm")
            scr_rc = dp.tile([NPARTS, BL], F32, name="scr_rc")

            # ---------------- BN folds (scoped scratch pool) ----------------
            with tc.tile_pool(name="foldtmp", bufs=1) as ftp:
                # per-layer folds: s = g/sqrt(rv+eps); t = (b_lin - rm)*s + be
                for li, (bnp, blin) in enumerate(((bn1, b1_p), (bn2, b2_p))):
                    st = ftp.tile([NPARTS, C], F32, tag="f_s", name=f"st{li}")
                    gt = ftp.tile([NPARTS, C], F32, tag="f_g", name=f"gt{li}")
                    tt = ftp.tile([NPARTS, C], F32, tag="f_t", name=f"tt{li}")
                    rmt = ftp.tile([NPARTS, C], F32, tag="f_rm", name=f"rmt{li}")
                    bet = ftp.tile([NPARTS, C], F32, tag="f_be", name=f"bet{li}")
                    nc.sync.dma_start(out=st[:], in_=bnp["rv"].rearrange("(p d) -> p d", d=C))
                    nc.sync.dma_start(out=gt[:], in_=bnp["g"].rearrange("(p d) -> p d", d=C))
                    nc.sync.dma_start(out=rmt[:], in_=bnp["rm"].rearrange("(p d) -> p d", d=C))
                    nc.sync.dma_start(out=bet[:], in_=bnp["be"].rearrange("(p d) -> p d", d=C))
                    nc.sync.dma_start(out=tt[:], in_=blin[None, :].to_broadcast([NPARTS, C]))
                    nc.vector.tensor_scalar_add(st[:], st[:], EPS)
                    nc.scalar.activation(out=st[:], in_=st[:], func=AF.Sqrt)
                    nc.vector.reciprocal(out=st[:], in_=st[:])
                    nc.vector.tensor_mul(st[:], st[:], gt[:])
                    nc.vector.tensor_sub(tt[:], tt[:], rmt[:])
                    nc.vector.tensor_mul(tt[:], tt[:], st[:])
                    nc.vector.tensor_add(tt[:], tt[:], bet[:])
                    nc.sync.dma_start(out=scr[li][0][:], in_=st[:])
                    nc.sync.dma_start(out=scr[li][1][:], in_=tt[:])

                # gn fold at (8, 2048)
                gnw = {}
                for k in ("g", "b", "rm", "rv"):
                    t = ftp.tile([BL, C], F32, tag=f"gn_{k}", name=f"gnl_{k}")
                    nc.sync.dma_start(out=t[:], in_=gn[k][None, :].to_broadcast([BL, C]))
                    gnw[k] = t
                nc.vector.tensor_scalar_add(sgn[:], gnw["rv"][:], EPS)
                nc.scalar.activation(out=sgn[:], in_=sgn[:], func=AF.Sqrt)
                nc.vector.reciprocal(out=sgn[:], in_=sgn[:])
                nc.vector.tensor_mul(sgn[:], sgn[:], gnw["g"][:])
                nc.vector.tensor_mul(tgn[:], gnw["rm"][:], sgn[:])
                nc.vector.tensor_sub(tgn[:], gnw["b"][:], tgn[:])

                # gb fold at (128, 16), c-minor layout: c = j*128 + p
                gbw = {}
                for k in ("g", "b", "rm", "rv"):
                    t = ftp.tile([128, KT], F32, tag=f"gb_{k}", name=f"gbl_{k}")
                    src = gb[k][:]
                    nc.sync.dma_start(
                        out=t[:],
                        in_=bass.AP(tensor=src.tensor, offset=src.offset,
                                    ap=[[1, 128], [128, KT]]))
                    gbw[k] = t
                nc.vector.tensor_scalar_add(sgb[:], gbw["rv"][:], EPS)
                nc.scalar.activation(out=sgb[:], in_=sgb[:], func=AF.Sqrt)
                nc.vector.reciprocal(out=sgb[:], in_=sgb[:])
                nc.vector.tensor_mul(sgb[:], sgb[:], gbw["g"][:])
                nc.vector.tensor_mul(tgb[:], gbw["rm"][:], sgb[:])
                nc.vector.tensor_sub(tgb[:], gbw["b"][:], tgb[:])
                nc.vector.tensor_scalar_mul(sgb[:], sgb[:], 1.0 / HW)  # GAP 1/HW

            # block-diag adj^T (72, 72)
            BD = cs.tile([R, R], F32)
            nc.gpsimd.memset(BD[:], 0.0)
            for b in range(BL):
                nc.sync.dma_start(
                    out=BD[NPARTS * b:NPARTS * (b + 1), NPARTS * b:NPARTS * (b + 1)],
                    in_=adj_p[b].transpose([1, 0]))
            BDr = cs.tile([R, R], DTM)
            nc.vector.tensor_copy(out=BDr[:], in_=BD[:])

            selfm98 = cs.tile([NPARTS, BL], F32)   # keep-mask per (part, image)
            rec98 = cs.tile([NPARTS, BL], F32)     # 1/(count+eps) per (part, image)
            mrec72 = cs.tile([R, 1], F32)          # mask branch row scale
            srec72 = cs.tile([R, 1], F32)          # self branch row scale
            G = cs.tile([128, BL, KT], F32)        # GAP sums, free=(b, j)
            Gn = cs.tile([128, BL, KT], F32)
            GT = cs.tile([128, 128], F32)

            with (
                tc.tile_pool(name="stream", bufs=3) as stream,
                tc.tile_pool(name="xtp", bufs=2) as xtp,
                tc.tile_pool(name="wp", bufs=5) as wp,
                tc.tile_pool(name="mm", bufs=1) as mm,
                tc.tile_pool(name="small", bufs=2) as sp,
            ):
                mfT = mm.tile([128, KT, R], DTM, tag="e")  # layer-1 lhsT (raw sums)

                # ---------------- per-image: mask, pooling, GAP ----------------
                for b in range(BL):
                    # mask rows -> (128,2) tile (partition = h*16+w, col = h-half)
                    msrc = mk_p[b, ::16, ::16]  # (16,16)
                    mr = sp.tile([128, 2], I32, tag="mr", name=f"mr{b}")
                    nc.sync.dma_start(out=mr[:, 0:1], in_=msrc[0:8, :])
                    nc.sync.dma_start(out=mr[:, 1:2], in_=msrc[8:16, :])
                    mrf = sp.tile([128, 2], F32, tag="mrf", name=f"mrf{b}")
                    nc.vector.tensor_copy(out=mrf[:], in_=mr[:])
                    oh = sp.tile([128, 2, NPARTS], DTM, tag="oh", name=f"oh{b}")
                    for h in range(2):
                        nc.vector.tensor_scalar(
                            out=oh[:, h, :], in0=iota_f[:], scalar1=mrf[:, h:h + 1],
                            scalar2=None, op0=OP.is_equal)

                    # counts -> rec / present / first-present -> selfmask
                    pcnt = ps.tile([NPARTS, 1], F32, tag="ps", name=f"pcnt{b}")
                    for h in range(2):
                        nc.tensor.matmul(pcnt[:], oh[:, h, :].bitcast(F32),
                                         ones_col[:],
                                         start=(h == 0), stop=(h == 1))
                    nc.vector.tensor_scalar_add(rec98[:, b:b + 1], pcnt[:], 1e-8)
                    nc.vector.reciprocal(out=rec98[:, b:b + 1], in_=rec98[:, b:b + 1])
                    pres = sp.tile([NPARTS, 1], F32, tag="pres", name=f"pres{b}")
                    nc.vector.tensor_scalar(out=pres[:], in0=pcnt[:], scalar1=0.0,
                                            scalar2=None, op0=OP.is_gt)
                    ppre = ps.tile([NPARTS, 1], F32, tag="ps", name=f"ppre{b}")
                    nc.tensor.matmul(ppre[:], L9[:], pres[:], start=True, stop=True)
                    isz = sp.tile([NPARTS, 1], F32, tag="isz", name=f"isz{b}")
                    nc.vector.tensor_scalar(out=isz[:], in0=ppre[:], scalar1=0.0,
                                            scalar2=None, op0=OP.is_equal)
                    nc.vector.tensor_mul(isz[:], isz[:], pres[:])
                    nc.vector.tensor_scalar(out=selfm98[:, b:b + 1], in0=isz[:],
                                            scalar1=-1.0, scalar2=1.0,
                                            op0=OP.mult, op1=OP.add)

                    # x_gcn load (c-minor: partition p = c%128, j = c//128)
                    xct = stream.tile([128, KT, HW], F32, tag="stream", name=f"xct{b}")
                    nc.sync.dma_start(
                        out=xct[:],
                        in_=xc_p[b].rearrange("(j p) hw -> p j hw", p=128))
                    # transpose to (hw, c) via PE
                    xt = xtp.tile([128, 2, C], DTM, tag="xt", name=f"xt{b}")
                    for j in range(KT):
                        for h in range(2):
                            ptt = ps.tile([128, 128], F32, tag="ps",
                                          name=f"ptt{b}_{j}_{h}")
                            nc.tensor.transpose(ptt[:], xct[:, j, 128 * h:128 * (h + 1)],
                                                ident[:])
                            nc.any.tensor_copy(out=xt[:, h, 128 * j:128 * (j + 1)],
                                               in_=ptt[:])
                    # segment raw sums: (9, 2048) = oh.T @ xt
                    mfeat = sp.tile([NPARTS, C], F32, tag="mfeat", name=f"mfeat{b}", bufs=1)
                    for n in range(NCH):
                        pseg = ps.tile([NPARTS, 512], F32, tag="ps", name=f"pseg{b}_{n}")
                        for h in range(2):
                            nc.tensor.matmul(pseg[:], oh[:, h, :],
                                             xt[:, h, 512 * n:512 * (n + 1)],
                                             start=(h == 0), stop=(h == 1))
                        nc.any.tensor_copy(out=mfeat[:, 512 * n:512 * (n + 1)],
                                           in_=pseg[:])
                    # transpose raw sums into layer-1 lhsT layout
                    for kt in range(KT):
                        ptm = ps.tile([128, NPARTS], F32, tag="ps", name=f"ptm{b}_{kt}")
                        nc.tensor.transpose(ptm[:], mfeat[:, 128 * kt:128 * (kt + 1)],
                                            ident[0:NPARTS, 0:NPARTS])
                        nc.any.tensor_copy(
                            out=mfT[:, kt, NPARTS * b:NPARTS * (b + 1)], in_=ptm[:])

                    # x_global load + GAP reduce
                    xgt = stream.tile([128, KT, HW], F32, tag="stream", name=f"xgt{b}")
                    nc.sync.dma_start(
                        out=xgt[:],
                        in_=xg_p[b].rearrange("(j p) hw -> p j hw", p=128))
                    nc.vector.reduce_sum(out=G[:, b, :], in_=xgt[:],
                                         axis=mybir.AxisListType.X)

                # row scales via DRAM bounce (transposed read to (72,1))
                nc.sync.dma_start(out=scr_sm[:], in_=selfm98[:])
                nc.sync.dma_start(out=scr_rc[:], in_=rec98[:])
                smv, rcv = scr_sm[:], scr_rc[:]
                nc.sync.dma_start(
                    out=mrec72[:],
                    in_=bass.AP(tensor=rcv.tensor, offset=rcv.offset,
                                ap=[[1, BL], [BL, NPARTS], [0, 1]]))
                nc.sync.dma_start(
                    out=srec72[:],
                    in_=bass.AP(tensor=smv.tensor, offset=smv.offset,
                                ap=[[1, BL], [BL, NPARTS], [0, 1]]))
                nc.vector.tensor_mul(srec72[:], srec72[:], mrec72[:])

                # ---------------- bnfeat_global output ----------------
                nc.vector.tensor_tensor(
                    Gn[:], G[:],
                    sgb[:, None, :].to_broadcast([128, BL, KT]), OP.mult)
                nc.vector.tensor_tensor(
                    Gn[:], Gn[:],
                    tgb[:, None, :].to_broadcast([128, BL, KT]), OP.add)
                pG = ps.tile([128, 128], F32, tag="ps")
                nc.tensor.transpose(pG[:], Gn[:].rearrange("p b j -> p (b j)"), ident[:])
                nc.any.tensor_copy(out=GT[:], in_=pG[:])
                nc.sync.dma_start(
                    out=out_p[:, 0:C].rearrange("b (j p) -> b j p", p=128), in_=GT[:])

                # layer-1 BN reps (72, 2048) via partition-broadcast reload
                srep = cs.tile([R, C], F32, tag="srep", name="srep1")
                trep = cs.tile([R, C], F32, tag="trep", name="trep1")
                nc.sync.dma_start(out=srep[:], in_=_bcast_part(scr[0][0][:], BL))
                nc.sync.dma_start(out=trep[:], in_=_bcast_part(scr[0][1][:], BL))

                # ---------------- GCN layer 1 ----------------
                s_all = mm.tile([R, C], DTM, tag="a")
                s_self = mm.tile([R, C], DTM, tag="b")
                psl1 = [ps.tile([R, 512], F32, tag="ps", name=f"psl1_{i}")
                        for i in range(NCH)]
                for ha in range(2):
                    for kt in range(KT):
                        w = wp.tile([128, C // 2], DTM, tag="w", name=f"w1_{ha}_{kt}")
                        if MM_FAST:  # SWDGE cast-DMA rounds f32 -> f32r in flight
                            nc.gpsimd.dma_start(
                                out=w[:],
                                in_=w1_p[128 * kt:128 * (kt + 1),
                                         1024 * ha:1024 * (ha + 1)])
                        else:
                            nc.sync.dma_start(
                                out=w[:],
                                in_=w1_p[128 * kt:128 * (kt + 1),
                                         1024 * ha:1024 * (ha + 1)])
                        for i in range(2):
                            n = 2 * ha + i
                            nc.tensor.matmul(psl1[n][:], mfT[:, kt, :],
                                             w[:, 512 * i:512 * (i + 1)],
                                             start=(kt == 0), stop=(kt == KT - 1))
                    for i in range(2):
                        n = 2 * ha + i
                        nc.vector.tensor_scalar(
                            out=s_all[:, 512 * n:512 * (n + 1)], in0=psl1[n][:],
                            scalar1=mrec72[:, 0:1], scalar2=None, op0=OP.mult)
                        nc.vector.tensor_scalar(
                            out=s_self[:, 512 * n:512 * (n + 1)], in0=psl1[n][:],
                            scalar1=srec72[:, 0:1], scalar2=None, op0=OP.mult)

                # bmm + BN1 + relu for both branches
                x1 = {}
                for br, s_in, xtag in (("m", s_all, "c"), ("s", s_self, "d")):
                    xo = mm.tile([R, C], F32, tag=xtag, name=f"x1{br}")
                    for n in range(NCH):
                        po = ps.tile([R, 512], F32, tag="ps", name=f"po1{br}{n}")
                        nc.tensor.matmul(po[:], BDr[:],
                                         s_in[:, 512 * n:512 * (n + 1)],
                                         start=True, stop=True)
                        sl = slice(512 * n, 512 * (n + 1))
                        nc.vector.tensor_tensor(xo[:, sl], po[:], srep[:, sl], OP.mult)
                        nc.vector.tensor_tensor(xo[:, sl], xo[:, sl], trep[:, sl], OP.add)
                        nc.scalar.activation(out=xo[:, sl], in_=xo[:, sl], func=AF.Relu)
                    x1[br] = xo

                # transpose x1 for layer 2
                x1T = {}
                for br, ttag in (("m", "e"), ("s", "f")):
                    xt1 = mm.tile([128, KT, R], DTM, tag=ttag, name=f"x1T{br}")
                    for kt in range(KT):
                        pt1 = ps.tile([128, R], F32, tag="ps", name=f"pt1{br}{kt}")
                        nc.tensor.transpose(pt1[:], x1[br][:, 128 * kt:128 * (kt + 1)],
                                            ident[0:R, 0:R])
                        nc.any.tensor_copy(out=xt1[:, kt, :], in_=pt1[:])
                    x1T[br] = xt1

                # ---------------- GCN layer 2 ----------------
                psl2 = {br: [ps.tile([R, 512], F32, tag="ps", name=f"psl2_{br}_{i}")
                             for i in range(NCH)] for br in ("m", "s")}
                for kt in range(KT):
                    w = wp.tile([128, C], DTM, tag="w", name=f"w2_{kt}")
                    if MM_FAST:
                        nc.sync.dma_start(out=w[:],
                                          in_=w2_p[128 * kt:128 * (kt + 1), :].bitcast(F32R))
                        nc.vector.tensor_copy(out=w[:], in_=w[:].bitcast(F32))
                    else:
                        nc.sync.dma_start(out=w[:], in_=w2_p[128 * kt:128 * (kt + 1), :])
                    for br in ("m", "s"):
                        for n in range(NCH):
                            nc.tensor.matmul(psl2[br][n][:], x1T[br][:, kt, :],
                                             w[:, 512 * n:512 * (n + 1)],
                                             start=(kt == 0), stop=(kt == KT - 1))
                s2 = {}
                for br, stag in (("m", "a"), ("s", "b")):
                    t = mm.tile([R, C], DTM, tag=stag, name=f"s2{br}")
                    for n in range(NCH):
                        nc.any.tensor_copy(out=t[:, 512 * n:512 * (n + 1)],
                                           in_=psl2[br][n][:])
                    s2[br] = t

                # layer-2 BN reps (reuse slots)
                srep2 = cs.tile([R, C], F32, tag="srep", name="srep2")
                trep2 = cs.tile([R, C], F32, tag="trep", name="trep2")
                nc.sync.dma_start(out=srep2[:], in_=_bcast_part(scr[1][0][:], BL))
                nc.sync.dma_start(out=trep2[:], in_=_bcast_part(scr[1][1][:], BL))

                # bmm + BN2 + relu -> x2 ; outputs
                cat_off = {"m": 3 * C, "s": 3 * C + NPARTS * C}
                bnf_off = {"m": C, "s": 2 * C}
                for br, xtag in (("m", "c"), ("s", "d")):
                    x2 = mm.tile([R, C], F32, tag=xtag, name=f"x2{br}")
                    boff = bnf_off[br]
                    off = cat_off[br]
                    catv = out_p[:, off:off + NPARTS * C].rearrange(
                        "b (p d) -> b p d", d=C)
                    # full-width mean staging tile (reuses the dead mfeat slot)
                    bnf = sp.tile([NPARTS, C], F32, tag="mfeat",
                                  name=f"bnf{br}", bufs=1)
                    for n in range(NCH):
                        po = ps.tile([R, 512], F32, tag="ps", name=f"po2{br}{n}")
                        nc.tensor.matmul(po[:], BDr[:],
                                         s2[br][:, 512 * n:512 * (n + 1)],
                                         start=True, stop=True)
                        sl = slice(512 * n, 512 * (n + 1))
                        nc.vector.tensor_tensor(x2[:, sl], po[:], srep2[:, sl], OP.mult)
                        nc.vector.tensor_tensor(x2[:, sl], x2[:, sl], trep2[:, sl], OP.add)
                        nc.scalar.activation(out=x2[:, sl], in_=x2[:, sl], func=AF.Relu)
                        # cat output chunk (drains while later chunks compute)
                        nc.sync.dma_start(out=catv[:, :, sl], in_=x2[:, sl])
                        # mean over parts + BN(gn) (exact fp32)
                        pf = ps.tile([BL, 512], F32, tag="ps", name=f"pf{br}{n}")
                        nc.tensor.matmul(pf[:], onesblk[:], x2[:, sl],
                                         start=True, stop=True)
                        nc.vector.tensor_tensor(bnf[0:BL, sl], pf[:], sgn[:, sl],
                                                OP.mult)
                        nc.vector.tensor_tensor(bnf[0:BL, sl], bnf[0:BL, sl],
                                                tgn[:, sl], OP.add)
                    nc.sync.dma_start(out=out_p[:, boff:boff + C], in_=bnf[0:BL, :])

    legalize_waits(nc)
    return nc


_CACHE = {}


def kernel(_run_kwargs=None, **inputs):
    run_kwargs = _run_kwargs or {}
    if "nc" not in _CACHE:
        _CACHE["nc"] = build_bass()
    nc = _CACHE["nc"]

    B = inputs["x_global"].shape[0]
    n_cores = 8
    bl = B // n_cores

    rep_names = ["W1", "W2", "b1", "b2", "g1", "be1", "rm1", "rv1",
                 "g2", "be2", "rm2", "rv2",
                 "gb_g", "gb_b", "gb_rm", "gb_rv",
                 "gn_g", "gn_b", "gn_rm", "gn_rv"]

    in_maps = []
    for c in range(n_cores):
        sl = slice(c * bl, (c + 1) * bl)
        m = {
            "x_global": np.ascontiguousarray(
                inputs["x_global"][sl]).reshape(bl, C, HW).astype(np.float32),
            "x_gcn": np.ascontiguousarray(
                inputs["x_gcn"][sl]).reshape(bl, C, HW).astype(np.float32),
            "mask": np.ascontiguousarray(
                inputs["mask"][sl, 0]).astype(np.int32),
            "adj": np.ascontiguousarray(inputs["adj"][sl]).astype(np.float32),
        }
        for k in rep_names:
            m[k] = np.ascontiguousarray(inputs[k]).astype(np.float32)
        in_maps.append(m)

    from concourse.bass_utils import run_bass_kernel_spmd
    res = run_bass_kernel_spmd(nc, in_maps, list(range(n_cores)), **run_kwargs)
    out = np.concatenate([res.results[c]["out"] for c in range(n_cores)], axis=0)
    _CACHE["last_results"] = res
    return out



# revision 11
# speedup vs baseline: 1.5975x; 1.5975x over previous
"""Trainium2 Bass kernel for nn_Baseline_SelfGCN (gnn_message_passing).

Data-parallel over batch: 8 NeuronCores x 8 images each. bf16 on device
(inputs/weights cast on host; PSUM accumulation stays f32), which halves
HBM traffic and doubles PE/DVE throughput. x_gcn is loaded pre-transposed
to (hw, c) layout by the DMA xbar (dma_start_transpose), so segment
pooling is a straight block matmul with a per-(image,h) one-hot operand
and no on-chip transposes of the activations.

Per core:
  - GAP of x_global (DVE reduce) + BN(gb)              -> out[:, 0:2048]
  - mask onehot -> counts/drop logic (batched over images)
  - segment raw sums for all 8 images into one (72, 2048) PSUM block;
    1/count and self-branch row masks are applied after the (linear)
    layer-1 matmul as row scalings
  - 2-layer GCN (x@W -> blockdiag(adjT)@s -> BN -> relu), both branches
    sharing the layer-1 raw matmul
  - mean over parts + BN(gn)                           -> out[:, 2048:6144]
  - concat features                                    -> out[:, 6144:43008]

Host side: shard/layout/dtype staging only (bf16 casts, BN param folds,
block-diag adj^T assembly, mask downsample/permute); all reductions and
matmuls run on device. Output is written bf16 and upcast on host.

Self-contained: hardcodes shapes; host side only shards/gathers.
"""

import numpy as np
import ml_dtypes

import concourse.bass as bass
import concourse.mybir as mybir
import concourse.tile as tile
from concourse.masks import make_identity

F32 = mybir.dt.float32
BF = mybir.dt.bfloat16
I32 = mybir.dt.int32
AF = mybir.ActivationFunctionType
OP = mybir.AluOpType

BL = 8          # images per core
C = 2048
HW = 256        # Hf*Wf
NP = 9          # graph nodes (parts 1..9)
R = BL * NP     # 72 rows = (image, part)
EPS = 1e-5
NCH = 4         # 2048 / 512 N-chunks
KT = 16         # 2048 / 128 K-tiles
OUTW = 3 * C + 2 * NP * C  # 43008
NPBF = ml_dtypes.bfloat16


def legalize_waits(nc, max_waits=1):
    """Split multi-wait instructions: this walrus build allows only one
    embedded sync-wait per instruction; hoist extras onto standalone
    InstEventSemaphore waits on the same engine."""
    cnt = 0
    for fn in nc.m.functions:
        for blk in fn.blocks:
            out = []
            for inst in blk.instructions:
                si = inst.sync_info
                if si is not None and si.on_wait and len(si.on_wait) > max_waits:
                    waits = list(si.on_wait)
                    for w in waits[:-max_waits]:
                        cnt += 1
                        wi = mybir.InstEventSemaphore(
                            name=f"wsplit{cnt}_{inst.name}", ins=[], outs=[],
                            sync_info=mybir.SyncInfo(on_wait=[w], on_update=[]))
                        wi.engine = inst.engine
                        nc.register_instruction(wi)
                        out.append(wi)
                    si.on_wait = waits[-max_waits:]
                    inst.sync_info = si
                out.append(inst)
            blk.instructions = out
    return cnt


def build_bass():
    nc = bass.Bass()

    xg_p = nc.declare_dram_parameter("xg", [BL, C, HW], BF, isOutput=False)
    xc_p = nc.declare_dram_parameter("xc", [BL, C, HW], BF, isOutput=False)
    mk_p = nc.declare_dram_parameter("mkp", [128, 2, BL], I32, isOutput=False)
    bd_p = nc.declare_dram_parameter("adjbd", [R, R], BF, isOutput=False)
    w1_p = nc.declare_dram_parameter("W1", [C, C], BF, isOutput=False)
    w2_p = nc.declare_dram_parameter("W2", [C, C], BF, isOutput=False)
    s1_p = nc.declare_dram_parameter("s1r", [R, C], BF, isOutput=False)
    t1_p = nc.declare_dram_parameter("t1r", [R, C], BF, isOutput=False)
    s2_p = nc.declare_dram_parameter("s2r", [R, C], BF, isOutput=False)
    t2_p = nc.declare_dram_parameter("t2r", [R, C], BF, isOutput=False)
    sgb_p = nc.declare_dram_parameter("sgb", [128, KT], BF, isOutput=False)
    tgb_p = nc.declare_dram_parameter("tgb", [128, KT], BF, isOutput=False)
    sgn_p = nc.declare_dram_parameter("sgn", [BL, C], BF, isOutput=False)
    tgn_p = nc.declare_dram_parameter("tgn", [BL, C], BF, isOutput=False)
    out_p = nc.declare_dram_parameter("out", [BL, OUTW], BF, isOutput=True)

    with tile.TileContext(nc) as tc:
        with (
            tc.tile_pool(name="consts", bufs=1) as cs,
            tc.tile_pool(name="ps", bufs=8, space="PSUM") as ps,
            tc.tile_pool(name="dram", bufs=1, space="DRAM") as dp,
        ):
            # ---------------- constants / params ----------------
            ident = cs.tile([128, 128], BF)
            make_identity(nc, ident[:])

            iota_i = cs.tile([128, NP], I32)
            nc.gpsimd.iota(iota_i[:], pattern=[[1, NP]], base=1,
                           channel_multiplier=0)
            iota_f = cs.tile([128, NP], BF)
            nc.gpsimd.tensor_copy(out=iota_f[:], in_=iota_i[:])

            ones_col = cs.tile([128, 1], BF)
            nc.gpsimd.memset(ones_col[:], 1.0)

            # strictly-lower-triangular L9: L[q,p] = 1 if q < p
            L9 = cs.tile([NP, NP], F32)
            nc.gpsimd.memset(L9[:], 0.0)
            nc.gpsimd.affine_select(
                out=L9[:], in_=L9[:], compare_op=OP.is_ge, fill=1.0,
                base=0, pattern=[[-1, NP]], channel_multiplier=1)

            # block "mean over parts" matrix (72, 8): 1/9 on image blocks
            onesblk = cs.tile([R, BL], BF)
            nc.gpsimd.memset(onesblk[:], 1.0 / NP)
            nc.gpsimd.affine_select(
                out=onesblk[:], in_=onesblk[:], compare_op=OP.is_ge, fill=0.0,
                base=0, pattern=[[-NP, BL]], channel_multiplier=1)
            nc.gpsimd.affine_select(
                out=onesblk[:], in_=onesblk[:], compare_op=OP.is_ge, fill=0.0,
                base=NP - 1, pattern=[[NP, BL]], channel_multiplier=-1)

            BDr = cs.tile([R, R], BF)
            nc.sync.dma_start(out=BDr[:], in_=bd_p[:, :])
            s1r = cs.tile([R, C], BF)
            t1r = cs.tile([R, C], BF)
            s2r = cs.tile([R, C], BF)
            t2r = cs.tile([R, C], BF)
            nc.sync.dma_start(out=s1r[:], in_=s1_p[:, :])
            nc.sync.dma_start(out=t1r[:], in_=t1_p[:, :])
            nc.sync.dma_start(out=s2r[:], in_=s2_p[:, :])
            nc.sync.dma_start(out=t2r[:], in_=t2_p[:, :])
            sgb = cs.tile([128, KT], BF)
            tgb = cs.tile([128, KT], BF)
            sgn = cs.tile([BL, C], BF)
            tgn = cs.tile([BL, C], BF)
            nc.sync.dma_start(out=sgb[:], in_=sgb_p[:, :])
            nc.sync.dma_start(out=tgb[:], in_=tgb_p[:, :])
            nc.sync.dma_start(out=sgn[:], in_=sgn_p[:, :])
            nc.sync.dma_start(out=tgn[:], in_=tgn_p[:, :])

            # DRAM scratch for (9,8) -> (72,1) repartition bounce
            scr_sm = dp.tile([NP, BL], F32, name="scr_sm")
            scr_rc = dp.tile([NP, BL], F32, name="scr_rc")

            # ---------------- mask -> onehot -> counts/drop ----------------
            mki = cs.tile([128, 2, BL], I32)
            nc.sync.dma_start(out=mki[:], in_=mk_p[:, :, :])
            mrf = cs.tile([128, 2, BL], F32)
            nc.vector.tensor_copy(out=mrf[:], in_=mki[:])
            oh = cs.tile([128, 2, BL, NP], BF)
            for h in range(2):
                for b in range(BL):
                    nc.vector.tensor_scalar(
                        out=oh[:, h, b, :], in0=iota_f[:],
                        scalar1=mrf[:, h, b:b + 1], scalar2=None,
                        op0=OP.is_equal)

            pc = ps.tile([NP, BL], F32, tag="ps", name="pc")
            for b in range(BL):
                for h in range(2):
                    nc.tensor.matmul(pc[:, b:b + 1], oh[:, h, b, :],
                                     ones_col[:], start=(h == 0), stop=(h == 1))
            rec98 = cs.tile([NP, BL], F32)
            nc.vector.tensor_scalar_add(rec98[:], pc[:], 1e-8)
            nc.vector.reciprocal(out=rec98[:], in_=rec98[:])
            pres = cs.tile([NP, BL], F32)
            nc.vector.tensor_scalar(out=pres[:], in0=pc[:], scalar1=0.0,
                                    scalar2=None, op0=OP.is_gt)
            ppre = ps.tile([NP, BL], F32, tag="ps", name="ppre")
            nc.tensor.matmul(ppre[:], L9[:], pres[:], start=True, stop=True)
            smrec = cs.tile([NP, BL], F32)
            nc.vector.tensor_scalar(out=smrec[:], in0=ppre[:], scalar1=0.0,
                                    scalar2=None, op0=OP.is_equal)
            nc.vector.tensor_mul(smrec[:], smrec[:], pres[:])
            # smrec = (1 - first_present) * rec
            nc.vector.tensor_scalar(out=smrec[:], in0=smrec[:], scalar1=-1.0,
                                    scalar2=1.0, op0=OP.mult, op1=OP.add)
            nc.vector.tensor_mul(smrec[:], smrec[:], rec98[:])
            nc.sync.dma_start(out=scr_rc[:], in_=rec98[:])
            nc.sync.dma_start(out=scr_sm[:], in_=smrec[:])
            mrec72 = cs.tile([R, 1], F32)
            srec72 = cs.tile([R, 1], F32)
            rcv, smv = scr_rc[:], scr_sm[:]
            nc.sync.dma_start(
                out=mrec72[:],
                in_=bass.AP(tensor=rcv.tensor, offset=rcv.offset,
                            ap=[[1, BL], [BL, NP], [0, 1]]))
            nc.sync.dma_start(
                out=srec72[:],
                in_=bass.AP(tensor=smv.tensor, offset=smv.offset,
                            ap=[[1, BL], [BL, NP], [0, 1]]))

            G = cs.tile([128, BL, KT], BF)      # GAP raw sums, free=(b, j)
            mfT = cs.tile([128, KT, R], BF)     # layer-1 lhsT (raw sums^T)

            with (
                tc.tile_pool(name="stream", bufs=3) as stream,
                tc.tile_pool(name="gstream", bufs=2) as gstream,
                tc.tile_pool(name="wp", bufs=3) as wp,
                tc.tile_pool(name="mm", bufs=1) as mm,
            ):
                # ---------- streaming: x_gcn transpose-loads + pooling,
                # ---------- x_global loads + GAP reduce
                # pooled raw sums land directly in lhsT layout (c, (b, p));
                # 2 j-chunks share each PSUM bank
                pj = [ps.tile([128, 2, R], F32, tag="ps", name=f"pj{q}")
                      for q in range(8)]
                for b in range(BL):
                    xcT = stream.tile([128, 2, C], BF, tag="xcT",
                                      name=f"xcT{b}")
                    nc.sync.dma_start_transpose(xcT[:], xc_p[b])
                    for q in range(8):
                        for jj in range(2):
                            j = 2 * q + jj
                            for h in range(2):
                                nc.tensor.matmul(
                                    pj[q][:, jj, NP * b:NP * (b + 1)],
                                    xcT[:, h, 128 * j:128 * (j + 1)],
                                    oh[:, h, b, :],
                                    start=(h == 0), stop=(h == 1))
                    if b % 2 == 0:
                        xgt = gstream.tile([128, 2, KT, HW], BF, tag="xg",
                                           name=f"xgt{b}")
                        nc.sync.dma_start(
                            out=xgt[:],
                            in_=xg_p[b:b + 2].rearrange(
                                "b (j p) hw -> p b j hw", p=128))
                        with nc.allow_low_precision("bf16 GAP accumulate"):
                            nc.vector.reduce_sum(
                                out=G[:, b:b + 2, :], in_=xgt[:],
                                axis=mybir.AxisListType.X)

                # ---------- bnfeat_global ----------
                Gn = cs.tile([128, BL, KT], BF)
                nc.vector.tensor_tensor(
                    Gn[:], G[:], sgb[:, None, :].to_broadcast([128, BL, KT]),
                    OP.mult)
                nc.vector.tensor_tensor(
                    Gn[:], Gn[:], tgb[:, None, :].to_broadcast([128, BL, KT]),
                    OP.add)
                pG = ps.tile([128, 128], BF, tag="ps", name="pG")
                nc.tensor.transpose(pG[:], Gn[:].rearrange("p b j -> p (b j)"),
                                    ident[:])
                GT = cs.tile([128, 128], BF)
                nc.scalar.activation(out=GT[:], in_=pG[:], func=AF.Copy)
                nc.sync.dma_start(
                    out=out_p[:, 0:C].rearrange("b (j p) -> b j p", p=128),
                    in_=GT[:])

                # ---------- pooled raw sums -> SBUF lhsT ----------
                for q in range(8):
                    if q % 2 == 0:
                        nc.scalar.activation(out=mfT[:, 2 * q:2 * (q + 1), :],
                                             in_=pj[q][:], func=AF.Copy)
                    else:
                        nc.vector.tensor_copy(out=mfT[:, 2 * q:2 * (q + 1), :],
                                              in_=pj[q][:])

                # ---------- GCN layer 1 ----------
                psl1 = [ps.tile([R, 512], F32, tag="ps", name=f"psl1_{n}")
                        for n in range(NCH)]
                for kp in range(8):
                    w = wp.tile([128, 2, C], BF, tag="w", name=f"w1_{kp}")
                    nc.sync.dma_start(
                        out=w[:],
                        in_=w1_p[256 * kp:256 * (kp + 1), :].rearrange(
                            "(t p) c -> p t c", p=128))
                    for t in range(2):
                        kt = 2 * kp + t
                        for n in range(NCH):
                            nc.tensor.matmul(
                                psl1[n][:], mfT[:, kt, :],
                                w[:, t, 512 * n:512 * (n + 1)],
                                start=(kt == 0), stop=(kt == KT - 1))

                s_all = mm.tile([R, C], BF, tag="sa")
                s_self = mm.tile([R, C], BF, tag="sb")
                for n in range(NCH):
                    sl = slice(512 * n, 512 * (n + 1))
                    nc.vector.tensor_scalar(
                        out=s_all[:, sl], in0=psl1[n][:],
                        scalar1=mrec72[:, 0:1], scalar2=None, op0=OP.mult)
                    nc.vector.tensor_scalar(
                        out=s_self[:, sl], in0=psl1[n][:],
                        scalar1=srec72[:, 0:1], scalar2=None, op0=OP.mult)

                # ---------- bmm + BN + relu (layer l, both branches) ------
                def bmm_bn_relu(s_in, sr, tr, xo, br):
                    for n in range(NCH):
                        sl = slice(512 * n, 512 * (n + 1))
                        po = ps.tile([R, 512], F32, tag="ps",
                                     name=f"po{br}{n}")
                        nc.tensor.matmul(po[:], BDr[:], s_in[:, sl],
                                         start=True, stop=True)
                        nc.vector.tensor_tensor(xo[:, sl], po[:],
                                                sr[:, sl], OP.mult)
                        nc.gpsimd.tensor_tensor(xo[:, sl], xo[:, sl],
                                                tr[:, sl], OP.add)
                        nc.scalar.activation(out=xo[:, sl], in_=xo[:, sl],
                                             func=AF.Relu)

                x1m = mm.tile([R, C], BF, tag="x1m")
                x1s = mm.tile([R, C], BF, tag="x1s")
                bmm_bn_relu(s_all, s1r, t1r, x1m, "m1")
                bmm_bn_relu(s_self, s1r, t1r, x1s, "s1")

                # ---------- transpose x1 for layer 2 ----------
                x1T = {}
                for br, x1 in (("m", x1m), ("s", x1s)):
                    xt1 = mm.tile([128, KT, R], BF, tag=f"x1T{br}")
                    for q in range(4):
                        ptr = ps.tile([128, 4, R], BF, tag="ps",
                                      name=f"pt1{br}{q}")
                        for t in range(4):
                            kt = 4 * q + t
                            nc.tensor.transpose(
                                ptr[:, t, :], x1[:, 128 * kt:128 * (kt + 1)],
                                ident[0:R, 0:R])
                        if q % 2 == 0:
                            nc.scalar.activation(
                                out=xt1[:, 4 * q:4 * (q + 1), :],
                                in_=ptr[:], func=AF.Copy)
                        else:
                            nc.vector.tensor_copy(
                                out=xt1[:, 4 * q:4 * (q + 1), :], in_=ptr[:])
                    x1T[br] = xt1

                # ---------- GCN layer 2 ----------
                psl2 = {br: [ps.tile([R, 512], F32, tag="ps",
                                     name=f"psl2_{br}_{n}")
                             for n in range(NCH)] for br in ("m", "s")}
                for kp in range(8):
                    w = wp.tile([128, 2, C], BF, tag="w", name=f"w2_{kp}")
                    nc.sync.dma_start(
                        out=w[:],
                        in_=w2_p[256 * kp:256 * (kp + 1), :].rearrange(
                            "(t p) c -> p t c", p=128))
                    for t in range(2):
                        kt = 2 * kp + t
                        for br in ("m", "s"):
                            for n in range(NCH):
                                nc.tensor.matmul(
                                    psl2[br][n][:], x1T[br][:, kt, :],
                                    w[:, t, 512 * n:512 * (n + 1)],
                                    start=(kt == 0), stop=(kt == KT - 1))

                s2m = mm.tile([R, C], BF, tag="sa")
                s2s = mm.tile([R, C], BF, tag="sb")
                for br, s2 in (("m", s2m), ("s", s2s)):
                    for n in range(NCH):
                        sl = slice(512 * n, 512 * (n + 1))
                        if n % 2 == 0:
                            nc.scalar.activation(out=s2[:, sl],
                                                 in_=psl2[br][n][:],
                                                 func=AF.Copy)
                        else:
                            nc.vector.tensor_copy(out=s2[:, sl],
                                                  in_=psl2[br][n][:])

                # ---------- layer-2 bmm + BN + relu; outputs ----------
                cat_off = {"m": 3 * C, "s": 3 * C + NP * C}
                bnf_off = {"m": C, "s": 2 * C}
                for br, s2 in (("m", s2m), ("s", s2s)):
                    x2 = mm.tile([R, C], BF, tag=f"x2{br}")
                    bmm_bn_relu(s2, s2r, t2r, x2, f"{br}2")
                    off = cat_off[br]
                    nc.sync.dma_start(
                        out=out_p[:, off:off + NP * C].rearrange(
                            "b (p d) -> b p d", d=C),
                        in_=x2[:])
                    # mean over parts + BN(gn)
                    bnf = mm.tile([BL, C], BF, tag=f"bnf{br}")
                    for n in range(NCH):
                        sl = slice(512 * n, 512 * (n + 1))
                        pf = ps.tile([BL, 512], F32, tag="ps",
                                     name=f"pf{br}{n}")
                        nc.tensor.matmul(pf[:], onesblk[:], x2[:, sl],
                                         start=True, stop=True)
                        nc.vector.tensor_tensor(bnf[:, sl], pf[:],
                                                sgn[:, sl], OP.mult)
                        nc.gpsimd.tensor_tensor(bnf[:, sl], bnf[:, sl],
                                                tgn[:, sl], OP.add)
                    boff = bnf_off[br]
                    nc.sync.dma_start(out=out_p[:, boff:boff + C], in_=bnf[:])

    legalize_waits(nc)
    return nc


_CACHE = {}


def _fold_bn(g, be, rm, rv, blin):
    s = g / np.sqrt(rv + EPS)
    t = (blin - rm) * s + be
    return s, t


def kernel(_run_kwargs=None, **inputs):
    run_kwargs = _run_kwargs or {}
    if "nc" not in _CACHE:
        _CACHE["nc"] = build_bass()
    nc = _CACHE["nc"]

    B = inputs["x_global"].shape[0]
    n_cores = 8
    bl = B // n_cores

    f = {k: np.asarray(inputs[k], np.float32) for k in inputs if k != "mask"}

    # BN folds (parameter preprocessing, replicated per core)
    s1, t1 = _fold_bn(f["g1"].reshape(NP, C), f["be1"].reshape(NP, C),
                      f["rm1"].reshape(NP, C), f["rv1"].reshape(NP, C),
                      f["b1"][None, :])
    s2, t2 = _fold_bn(f["g2"].reshape(NP, C), f["be2"].reshape(NP, C),
                      f["rm2"].reshape(NP, C), f["rv2"].reshape(NP, C),
                      f["b2"][None, :])
    s1r = np.tile(s1, (BL, 1)).astype(NPBF)
    t1r = np.tile(t1, (BL, 1)).astype(NPBF)
    s2r = np.tile(s2, (BL, 1)).astype(NPBF)
    t2r = np.tile(t2, (BL, 1)).astype(NPBF)
    sgb_ = f["gb_g"] / np.sqrt(f["gb_rv"] + EPS)
    tgb_ = f["gb_b"] - f["gb_rm"] * sgb_
    sgb = np.ascontiguousarray((sgb_ / HW).reshape(KT, 128).T).astype(NPBF)
    tgb = np.ascontiguousarray(tgb_.reshape(KT, 128).T).astype(NPBF)
    sgn_ = f["gn_g"] / np.sqrt(f["gn_rv"] + EPS)
    tgn_ = f["gn_b"] - f["gn_rm"] * sgn_
    sgn = np.tile(sgn_[None, :], (BL, 1)).astype(NPBF)
    tgn = np.tile(tgn_[None, :], (BL, 1)).astype(NPBF)
    w1 = f["W1"].astype(NPBF)
    w2 = f["W2"].astype(NPBF)

    mask_ds = np.asarray(inputs["mask"])[:, 0, ::16, ::16]  # (B, 16, 16)

    in_maps = []
    for c in range(n_cores):
        sl = slice(c * bl, (c + 1) * bl)
        # mask (bl,16,16) -> [p, h, b] with hw = 128*h + p
        md = mask_ds[sl].reshape(bl, 256).T.reshape(2, 128, bl)
        adj = f["adj"][sl]
        bd = np.zeros((R, R), np.float32)
        for b in range(bl):
            bd[NP * b:NP * (b + 1), NP * b:NP * (b + 1)] = adj[b].T
        m = {
            "xg": np.ascontiguousarray(f["x_global"][sl]).reshape(
                bl, C, HW).astype(NPBF),
            "xc": np.ascontiguousarray(f["x_gcn"][sl]).reshape(
                bl, C, HW).astype(NPBF),
            "mkp": np.ascontiguousarray(md.transpose(1, 0, 2)).astype(np.int32),
            "adjbd": bd.astype(NPBF),
            "W1": w1, "W2": w2,
            "s1r": s1r, "t1r": t1r, "s2r": s2r, "t2r": t2r,
            "sgb": sgb, "tgb": tgb, "sgn": sgn, "tgn": tgn,
        }
        in_maps.append(m)

    from concourse.bass_utils import run_bass_kernel_spmd
    res = run_bass_kernel_spmd(nc, in_maps, list(range(n_cores)), **run_kwargs)
    out = np.concatenate(
        [np.asarray(res.results[c]["out"]).astype(np.float32)
         for c in range(n_cores)], axis=0)
    _CACHE["last_results"] = res
    return out
